# revision 7
# baseline (speedup 1.0000x reference)
"""Trainium2 Bass kernel for nn_Net_76622216561354 (gnn_message_passing).

Self-contained: host-side sharding/index prep (numpy) + an 8-core SPMD
Bass/Tile kernel run via run_bass_kernel_spmd. Accepts FULL inputs, returns
the FULL pooled output [8192] float32.
"""
import numpy as np
import concourse.bass as bass
import concourse.mybir as mybir
import concourse.tile as tile
from concourse import bacc
from contextlib import ExitStack
import os

import numpy as np

NC = 8
N = 131072; E = 524288; F_IN = 16; DIM = 64; DNN = 16; BK = 4; NG = 8192
NL1 = 4; NL2 = 2
SUB = 2112
NPAD = 8 * SUB          # 16896
HALF = NPAD // 2        # 8448
ECH = 512
GSLOT = 192             # pooled graph slots per sub-chunk (padded)


def host_prep(inputs):
    ei = np.asarray(inputs["edge_index"])
    batch = np.asarray(inputs["batch"]).astype(np.int64)
    src, dst = ei[0].astype(np.int64), ei[1].astype(np.int64)

    # ---- graph spans ----
    # graphs may be empty; gstart[g] = first node of graph g (batch sorted)
    gsizes = np.bincount(batch, minlength=NG)
    gstart = np.concatenate([[0], np.cumsum(gsizes)])

    # ---- core cuts at graph boundaries ----
    cuts = [0]
    for c in range(1, NC):
        t = c * (N // NC)
        while t < N and batch[t] == batch[t - 1]:
            t += 1
        cuts.append(t)
    cuts.append(N)
    cuts = np.array(cuts, np.int64)

    # ---- per-core: pack graphs into 8 graph-aligned sub-chunks ----
    g2l = np.full(N, -1, np.int64)      # global node -> local slot (within its core)
    node_core = np.zeros(N, np.int64)
    l2g = [np.full(NPAD, -1, np.int64) for _ in range(NC)]
    # pooling bookkeeping: per core, per sub-chunk: list of (graph_id, end_pos)
    pool_graphs = [[[] for _ in range(8)] for _ in range(NC)]
    pool_mask = [np.zeros((8, SUB), np.float32) for _ in range(NC)]

    for c in range(NC):
        lo, hi = cuts[c], cuts[c + 1]
        glo, ghi = batch[lo], (batch[hi - 1] + 1 if hi > lo else batch[lo])
        s = 0; pos = 0
        for g in range(glo, ghi):
            sz = int(gsizes[g])
            if sz == 0:
                continue
            if pos + sz > SUB:
                s += 1; pos = 0
                assert s < 8, f"core {c}: sub-chunk overflow"
                assert sz <= SUB
            nodes = np.arange(gstart[g], gstart[g] + sz)
            slots = s * SUB + pos + np.arange(sz)
            g2l[nodes] = slots
            node_core[nodes] = c
            l2g[c][slots] = nodes
            pool_mask[c][s, pos + 1: pos + sz] = 1.0  # same-graph continuation
            # pos of graph end within sub-chunk stream
            pool_graphs[c][s].append((g, pos + sz - 1))
            pos += sz
        assert hi == lo or batch[hi - 1] + 1 == ghi

    # ---- per (core, block) edge streams ----
    # count first to get EP
    counts = np.zeros((NC, NC), np.int64)
    dst_core = node_core[dst]; src_core = node_core[src]
    for c in range(NC):
        for b in range(NC):
            counts[c, b] = np.count_nonzero((dst_core == c) & (src_core == b))
    maxcnt = int(counts.max())
    EP = ((maxcnt + 1 + ECH - 1) // ECH) * ECH
    nchunk = EP // ECH

    dstslot = g2l[dst]; srcslot = g2l[src]

    indeg = np.bincount(dst, minlength=N).astype(np.float64)
    inv = 1.0 / np.maximum(indeg, 1.0)
    ea_all = np.asarray(inputs["edge_attr"]).astype(np.float64)

    per_core = []
    for c in range(NC):
        gidx = np.zeros((8, EP), np.int64)       # src local slot per stream pos
        craw = np.zeros((8, 6, EP), np.float32)  # inv, inv*ea*4, mask (premult)
        ends = np.zeros((8, NPAD), np.int64)
        for b in range(NC):
            m = (dst_core == c) & (src_core == b)
            eids = np.nonzero(m)[0]
            order = np.argsort(dstslot[eids], kind="stable")
            eids = eids[order]
            k = len(eids)
            ps = 1 + np.arange(k)               # positions (0 = dummy)
            gidx[b, ps] = srcslot[eids]
            einv = inv[dst[eids]]
            craw[b, 0, ps] = einv.astype(np.float32)
            for q in range(4):
                craw[b, 1 + q, ps] = (einv * ea_all[eids, q]).astype(np.float32)
            dsl = dstslot[eids]
            same = np.zeros(k, bool)
            if k > 0:
                same[1:] = dsl[1:] == dsl[:-1]
                craw[b, 5, ps] = same.astype(np.float32)
                last = np.zeros(NPAD, np.int64)
                last[dsl] = ps                  # dsl sorted -> last write wins
                ends[b] = last
        per_core.append(dict(gidx=gidx, craw=craw, ends=ends))

    # ---- wrap helper: seq -> [16, L/16] with idx[p, s] = seq[s*16+p] ----
    def wrap16(seq):
        L = len(seq)
        assert L % 16 == 0
        return np.asarray(seq).reshape(L // 16, 16).T.copy()

    ECHUNKS = [4096, 4096, 4096, 4096, 512]
    assert sum(ECHUNKS) == NPAD

    in_maps = []
    for c in range(NC):
        pc = per_core[c]
        # gather idx: [128, EP/16] int16, wrapped per gather call
        # (super-chunks of 4*ECH, remainder as one final call)
        GCH = 4 * ECH
        gidx_t = np.zeros((128, EP // 16), np.int16)
        for b in range(NC):
            off = 0
            while off < EP:
                L = min(GCH, EP - off)
                seq = pc["gidx"][b, off:off + L]
                gidx_t[16 * b:16 * (b + 1), off // 16:(off + L) // 16] = \
                    wrap16(seq).astype(np.int16)
                off += L
        # ends idx: [128, NPAD/16] int16, wrapped per ends-chunk
        eidx_t = np.zeros((128, NPAD // 16), np.int16)
        off = 0
        for L in ECHUNKS:
            for b in range(NC):
                seq = pc["ends"][b, off:off + L]
                eidx_t[16 * b:16 * (b + 1), off // 16:(off + L) // 16] = \
                    wrap16(seq).astype(np.int16)
            off += L
        # c compact (premultiplied, NOT replicated): [8, 6, EP] fp16
        craw8 = pc["craw"].astype(np.float16)
        # pooling mask compact: [8, SUB] fp16
        pmask8 = pool_mask[c].astype(np.float16)
        pidx_t = np.zeros((128, GSLOT // 16), np.int16)
        for s in range(8):
            seq = np.zeros(GSLOT, np.int64)
            gl = pool_graphs[c][s]
            assert len(gl) <= GSLOT, f"GSLOT overflow: {len(gl)}"
            for i, (g, endpos) in enumerate(gl):
                seq[i] = endpos
            pidx_t[16 * s:16 * (s + 1), :] = wrap16(seq).astype(np.int16)
        # x slab transposed [16, NPAD] f16
        xT = np.zeros((16, NPAD), np.float16)
        real = l2g[c] >= 0
        xT[:, real] = np.asarray(inputs["x"])[l2g[c][real]].T.astype(np.float16)
        in_maps.append(dict(xT=xT, gidx=gidx_t, eidx=eidx_t, craw8=craw8,
                            pmask8=pmask8, pidx=pidx_t))

    meta = dict(EP=EP, nchunk=nchunk, ECHUNKS=ECHUNKS, cuts=cuts,
                pool_graphs=pool_graphs, l2g=l2g)
    return in_maps, meta


def fold_weights_host(inputs):
    """float64 weight folds -> shipped stationaries/biases (per-core identical)."""
    dt = np.float64
    lin0_w = np.asarray(inputs["lin0_w"], dt); lin0_b = np.asarray(inputs["lin0_b"], dt)
    lin1_w = np.asarray(inputs["lin1_w"], dt); lin1_b = np.asarray(inputs["lin1_b"], dt)
    lin2_w = np.asarray(inputs["lin2_w"], dt)
    root_w = np.asarray(inputs["root_w"], dt); conv_b = np.asarray(inputs["conv_b"], dt)
    nn1_w = np.asarray(inputs["nn1_w"], dt); nn1_b = np.asarray(inputs["nn1_b"], dt)
    gw_ih = np.asarray(inputs["gru_w_ih"], dt); gw_hh = np.asarray(inputs["gru_w_hh"], dt)
    gb_ih = np.asarray(inputs["gru_b_ih"], dt); gb_hh = np.asarray(inputs["gru_b_hh"], dt)

    Bm = nn1_b.reshape(DNN, DNN)
    Ak = nn1_w.reshape(BK, DNN, DNN)
    M = np.concatenate([Bm[None], Ak], axis=0)            # [5,16,16]

    w = {}
    # compact shipped forms; block-diag / tiled stationaries are expanded
    # on-device with small broadcast DMAs (keeps the relay payload tiny).
    w["lin1w"] = lin1_w.astype(np.float16)                 # [64, 16]
    w["M5"] = M.astype(np.float16)                         # [5, 16, 16]
    whs64 = np.zeros((NL1 * 4, 64, 64), np.float32)
    wfold16 = np.zeros((NL1 * 3, 16, 64), np.float32)
    biases = np.zeros((128, 17), np.float32)
    for j in range(NL1):
        P = lin1_w @ root_w @ gw_ih[j].T                  # [64,192]
        W_rz = P[:, :2 * DIM] + gw_hh[j].T[:, :2 * DIM]
        W_ni = P[:, 2 * DIM:]
        W_nh = gw_hh[j].T[:, 2 * DIM:]
        grp_w = [W_rz[:, :64], W_rz[:, 64:], W_ni, W_nh]
        for g in range(4):
            whs64[4 * j + g] = grp_w[g]
        wihT = gw_ih[j].T                                  # [16,192]
        for g in range(3):
            wfold16[3 * j + g] = wihT[:, 64 * g:64 * (g + 1)]
        b_base = (lin1_b @ root_w + conv_b) @ gw_ih[j].T   # [192]
        b_rz = b_base[:2 * DIM] + gb_ih[j][:2 * DIM] + gb_hh[j][:2 * DIM]
        b_ni = b_base[2 * DIM:] + gb_ih[j][2 * DIM:]
        b_hn = gb_hh[j][2 * DIM:]
        vec = [b_rz[:64], b_rz[64:], b_ni, b_hn]
        for g in range(4):
            biases[0:64, 4 * j + g] = vec[g]
            biases[64:128, 4 * j + g] = vec[g]
    w["whs64"] = whs64.astype(np.float16)
    w["wfold16"] = wfold16.astype(np.float16)
    biases[0:64, 16] = lin0_b
    biases[64:128, 16] = lin0_b
    w["biases"] = biases.astype(np.float32)
    w["lin0c"] = lin0_w.astype(np.float16)                 # [16, 64]
    # y stationary [128, 2] f16
    wy = np.zeros((128, 2), np.float32)
    wy[0:64, 0] = lin2_w[:, 0]
    wy[64:128, 1] = lin2_w[:, 0]
    w["wy"] = wy.astype(np.float16)
    return w


# ================= kernel builder =================

import concourse.bass as bass
import concourse.mybir as mybir
import concourse.tile as tile
from concourse import bacc
from contextlib import ExitStack

NITER = 8

f32 = mybir.dt.float32
f16 = mybir.dt.float16
i16 = mybir.dt.int16
AF = mybir.ActivationFunctionType
OP = mybir.AluOpType


def pieces(total, step):
    out = []
    off = 0
    while off < total:
        out.append((off, min(step, total - off)))
        off += step
    return out


def ends_pieces(c0, L):
    out = []
    while L > 0:
        ch = c0 // SUB
        off = c0 % SUB
        ln = min(L, SUB - off)
        out.append((ch, off, ln))
        c0 += ln
        L -= ln
    return out


def build(EP, fake_collective=False, niter=NITER):
    nchunk = EP // ECH
    nc = bacc.Bacc("TRN2", target_bir_lowering=False, debug=False, num_devices=NC)

    xT_d = nc.dram_tensor("xT", [16, NPAD], f16, kind="ExternalInput")
    gidx_d = nc.dram_tensor("gidx", [128, EP // 16], i16, kind="ExternalInput")
    eidx_d = nc.dram_tensor("eidx", [128, NPAD // 16], i16, kind="ExternalInput")
    craw8_d = nc.dram_tensor("craw8", [8, 6, EP], f16, kind="ExternalInput")
    pmask8_d = nc.dram_tensor("pmask8", [8, SUB], f16, kind="ExternalInput")
    pidx_d = nc.dram_tensor("pidx", [128, GSLOT // 16], i16, kind="ExternalInput")
    lin1w_d = nc.dram_tensor("lin1w", [64, 16], f16, kind="ExternalInput")
    M5_d = nc.dram_tensor("M5", [5, 16, 16], f16, kind="ExternalInput")
    whs64_d = nc.dram_tensor("whs64", [NL1 * 4, 64, 64], f16, kind="ExternalInput")
    wfold16_d = nc.dram_tensor("wfold16", [NL1 * 3, 16, 64], f16,
                               kind="ExternalInput")
    biases_d = nc.dram_tensor("biases", [128, 17], f32, kind="ExternalInput")
    lin0c_d = nc.dram_tensor("lin0c", [16, 64], f16, kind="ExternalInput")
    wy_d = nc.dram_tensor("wy", [128, 2], f16, kind="ExternalInput")
    out_d = nc.dram_tensor("pooled", [8, GSLOT], f32, kind="ExternalOutput")

    PIECES_H = pieces(HALF, 512)
    PIECES_S = pieces(SUB, 512)

    with tile.TileContext(nc) as tc, ExitStack() as ex:
        pp = ex.enter_context(tc.tile_pool(name="persist", bufs=1))
        wk = ex.enter_context(tc.tile_pool(name="work", bufs=2))
        wk2 = ex.enter_context(tc.tile_pool(name="work2", bufs=2))
        ps = ex.enter_context(tc.tile_pool(name="psum", bufs=8, space="PSUM"))
        dr = ex.enter_context(tc.tile_pool(name="dram", bufs=1, space="DRAM"))

        BUFA = dict(tag="bufA")   # >= 8.25KB slots
        BUFB = dict(tag="bufB")
        GGT = dict(tag="gg")      # 2KB slots

        hT = pp.tile([128, HALF], f16, tag="hT")
        table = pp.tile([128, NPAD, 2], f16, tag="table")
        cum = pp.tile([128, EP, 2], f16, tag="cum")
        nc.vector.memset(cum[:], 0)
        gidx = pp.tile([128, EP // 16], i16, tag="gidx")
        eidx = pp.tile([128, NPAD // 16], i16, tag="eidx")
        pmask = pp.tile([128, SUB], f16, tag="pmask")
        pidx = pp.tile([128, GSLOT // 16], i16, tag="pidx")
        biases = pp.tile([128, 17], f32, tag="biases")
        wy = pp.tile([128, 2], f16, tag="wy")

        nc.sync.dma_start(out=gidx[:], in_=gidx_d[:])
        nc.sync.dma_start(out=eidx[:], in_=eidx_d[:])
        # pmask broadcast-expand [8,SUB] -> [128,SUB] (each row 16x)
        nc.sync.dma_start(
            out=pmask[:], in_=bass.AP(pmask8_d, 0, [(SUB, 8), (0, 16), (1, SUB)]))
        nc.sync.dma_start(out=pidx[:], in_=pidx_d[:])
        nc.sync.dma_start(out=biases[:], in_=biases_d[:])
        nc.sync.dma_start(out=wy[:], in_=wy_d[:])

        # ---- stationaries expanded on-device from compact shipped forms ----
        # wslab: block b holds lin1_w at partitions 64*(b//4).., cols 16b..
        wslab_s = pp.tile([128, 8, 128], f16, tag="wslab_s")
        nc.vector.memset(wslab_s[:], 0)
        PS = wslab_s[:].ap[0][0]
        for grp in range(2):
            dst = bass.AP(wslab_s.tensor,
                          wslab_s[:].offset + grp * (64 * PS + 4 * 128 + 64),
                          [(PS, 64), (128 + 16, 4), (1, 16)])
            nc.sync.dma_start(
                out=dst, in_=bass.AP(lin1w_d, 0, [(16, 64), (0, 4), (1, 16)]))
        # wM: block-diag-8 of M[p] per stream p
        wM_s = pp.tile([128, 5, 128], f16, tag="wM_s")
        nc.vector.memset(wM_s[:], 0)
        PSM = wM_s[:].ap[0][0]
        for b in range(8):
            dst = bass.AP(wM_s.tensor, wM_s[:].offset + 16 * b * PSM + 16 * b,
                          [(PSM, 16), (128, 5), (1, 16)])
            nc.sync.dma_start(
                out=dst, in_=bass.AP(M5_d, 0, [(16, 16), (256, 5), (1, 16)]))
        # whs: two diagonal 64x64 copies per (j,g)
        whs_s = pp.tile([128, NL1 * 4, 128], f16, tag="whs_s")
        nc.vector.memset(whs_s[:], 0)
        PSH = whs_s[:].ap[0][0]
        for h in range(2):
            dst = bass.AP(whs_s.tensor, whs_s[:].offset + h * (64 * PSH + 64),
                          [(PSH, 64), (128, 16), (1, 64)])
            nc.sync.dma_start(
                out=dst, in_=bass.AP(whs64_d, 0, [(64, 64), (4096, 16), (1, 64)]))
        # wfold: [16,64] block tiled 8x down the partitions
        wfold_s = pp.tile([128, NL1 * 3, 64], f16, tag="wfold_s")
        PSF = wfold_s[:].ap[0][0]
        for b in range(8):
            dst = bass.AP(wfold_s.tensor, wfold_s[:].offset + 16 * b * PSF,
                          [(PSF, 16), (64, 12), (1, 64)])
            nc.sync.dma_start(
                out=dst, in_=bass.AP(wfold16_d, 0, [(64, 16), (1024, 12), (1, 64)]))

        slab_dram = dr.tile([128, SUB * 2], f16)
        ag_dram = dr.tile([NC, 128, SUB * 2], f16)
        cexp_dram = dr.tile([nchunk, 128, 6 * ECH], f16)
        y_dram = dr.tile([2, HALF], f32)

        # ================= INIT =================
        # expand compact craw8 [8,6,EP] -> chunked cexp_dram [nchunk,128,6*ECH]
        # (DRAM->DRAM, each block row duplicated over its 16 partitions)
        for k in range(nchunk):
            for s in range(6):
                dst = bass.AP(cexp_dram.tensor,
                              cexp_dram[:].offset + k * (128 * 6 * ECH) + s * ECH,
                              [(6 * ECH, 128), (1, ECH)])
                src = bass.AP(craw8_d, s * EP + k * ECH,
                              [(6 * EP, 8), (0, 16), (1, ECH)])
                nc.sync.dma_start(out=dst, in_=src)

        # lin0 -> hT
        wlin0 = pp.tile([16, 2, 128], f16, tag="wlin0")
        nc.vector.memset(wlin0[:], 0)
        PSL = wlin0[:].ap[0][0]
        for h in range(2):
            dst = bass.AP(wlin0.tensor, wlin0[:].offset + h * (128 + 64),
                          [(PSL, 16), (1, 64)])
            nc.sync.dma_start(
                out=dst, in_=bass.AP(lin0c_d, 0, [(64, 16), (1, 64)]))
        for c0, L in PIECES_H:
            xa = wk2.tile([16, 512], f16, **GGT)
            nc.sync.dma_start(out=xa[:, :L], in_=xT_d[:, c0:c0 + L])
            xb = wk2.tile([16, 512], f16, **GGT)
            nc.sync.dma_start(out=xb[:, :L], in_=xT_d[:, HALF + c0:HALF + c0 + L])
            p0 = ps.tile([128, 512], f32, tag="ps")
            nc.tensor.matmul(p0[:, :L], wlin0[:, 0, :], xa[:, :L],
                             start=True, stop=False)
            nc.tensor.matmul(p0[:, :L], wlin0[:, 1, :], xb[:, :L],
                             start=False, stop=True)
            nc.scalar.activation(out=hT[:, c0:c0 + L], in_=p0[:, :L],
                                 func=AF.Relu, bias=biases[:, 16:17], scale=1.0)

        # ================= ITERATIONS =================
        for it in range(niter):
            j = (it // 2) % NL1
            # ---- A: slab + exchange ----
            for c0, L in PIECES_S:
                p0 = ps.tile([128, 512], f32, tag="ps")
                for b in range(8):
                    rc0 = (b % 4) * SUB + c0
                    nc.tensor.matmul(p0[:, :L], wslab_s[:, b, :],
                                     hT[:, rc0:rc0 + L],
                                     start=(b == 0), stop=(b == 7))
                stg = wk2.tile([128, 1024], f16, tag="slabstg")
                for dup in range(2):
                    dst = bass.AP(stg.tensor, stg[:].offset + dup,
                                  [stg[:].ap[0], (2, L)])
                    nc.vector.tensor_copy(out=dst, in_=p0[:, :L])
                nc.sync.dma_start(out=slab_dram[:, c0 * 2:(c0 + L) * 2],
                                  in_=stg[:, :2 * L])
            if fake_collective:
                for cc_ in range(NC):
                    nc.sync.dma_start(out=ag_dram[cc_], in_=slab_dram[:])
            else:
                nc.gpsimd.collective_compute(
                    "AllGather", OP.bypass,
                    replica_groups=[list(range(NC))],
                    ins=[slab_dram[:].opt()], outs=[ag_dram[:].opt()])
            for s in range(8):
                src = bass.AP(ag_dram.tensor,
                              ag_dram[:].offset + (16 * s) * (SUB * 2),
                              [(128 * SUB * 2, 8), (SUB * 2, 16), (1, SUB * 2)])
                dst = bass.AP(table.tensor, table[:].offset + s * SUB * 2,
                              [table[:].ap[0], (1, SUB * 2)])
                nc.sync.dma_start(out=dst, in_=src)

            # ---- C: edge chunks (gathers batched 4x to amortize ap_gather) ----
            GCH = 4 * ECH
            gbuf = None
            for k in range(nchunk):
                cc = wk.tile([128, 6, ECH], f16, **BUFA)
                nc.sync.dma_start(out=cc[:], in_=cexp_dram[k])
                if k % 4 == 0:
                    G0 = k * ECH
                    GL = min(GCH, EP - G0)
                    gbuf = pp.tile([128, GCH, 2], f16, tag="gbuf")
                    nc.gpsimd.ap_gather(
                        out_ap=gbuf[:, :GL, :], in_ap=table[:],
                        idxs_ap=gidx[:, G0 // 16:(G0 + GL) // 16],
                        channels=128, num_elems=NPAD, d=2, num_idxs=GL)
                sc = wk.tile([128, 5, ECH], f16, **BUFB)
                g_in0 = bass.AP(gbuf.tensor,
                                gbuf[:].offset + (k % 4) * ECH * 2,
                                [gbuf[:].ap[0], (0, 5), (2, ECH)])
                nc.vector.tensor_tensor(out=sc[:], in0=g_in0, in1=cc[:, 0:5, :],
                                        op=OP.mult)
                msg = ps.tile([128, 512], f32, tag="ps")
                for p in range(5):
                    nc.tensor.matmul(msg[:, :ECH], wM_s[:, p, :], sc[:, p, :],
                                     start=(p == 0), stop=(p == 4))
                cum_out = bass.AP(cum.tensor, cum[:].offset + k * ECH * 2,
                                  [cum[:].ap[0], (2, ECH)])
                if k == 0:
                    init = 0.0
                else:
                    init = bass.AP(cum.tensor, cum[:].offset + (k * ECH - 1) * 2,
                                   [cum[:].ap[0], (1, 1)])
                nc.vector.tensor_tensor_scan(out=cum_out, data0=cc[:, 5, :],
                                             data1=msg[:, :ECH], initial=init,
                                             op0=OP.mult, op1=OP.add)

            # ---- D+E: ends + gates ----
            ends_tiles = {}

            def get_ends(ch, _et=ends_tiles):
                if ch in _et:
                    return _et[ch]
                eb = wk.tile([128, SUB, 2], f16, **(BUFA if ch < 4 else BUFB))
                nc.gpsimd.ap_gather(
                    out_ap=eb[:], in_ap=cum[:],
                    idxs_ap=eidx[:, ch * (SUB // 16):(ch + 1) * (SUB // 16)],
                    channels=128, num_elems=EP, d=2, num_idxs=SUB)
                for o in list(_et):
                    if o != ch and (o < 4) == (ch < 4):
                        del _et[o]
                _et[ch] = eb
                return eb

            for c0, L in PIECES_H:
                pr = ps.tile([128, 512], f32, tag="ps")
                pz = ps.tile([128, 512], f32, tag="ps")
                pn = ps.tile([128, 512], f32, tag="ps")
                ph = ps.tile([128, 512], f32, tag="ps")
                for g, pst in enumerate([pr, pz, pn, ph]):
                    nc.tensor.matmul(pst[:, :L], whs_s[:, 4 * j + g, :],
                                     hT[:, c0:c0 + L],
                                     start=True, stop=True)
                for g, pst in enumerate([pr, pz, pn]):
                    for half in range(2):
                        pcs = ends_pieces(half * HALF + c0, L)
                        for i, (ch, eoff, eln) in enumerate(pcs):
                            eb = get_ends(ch)
                            rhs = bass.AP(eb.tensor, eb[:].offset + eoff * 2,
                                          [eb[:].ap[0], (2, eln)])
                            oo = (eoff + ch * SUB) - (half * HALF + c0)
                            out = bass.AP(
                                pst.tensor,
                                pst[:].offset + 64 * half * pst[:].ap[0][0] + oo,
                                [(pst[:].ap[0][0], 64), (1, eln)])
                            tp = (0, 64) if half == 1 else None
                            nc.tensor.matmul(out, wfold_s[:, 3 * j + g, :], rhs,
                                             start=False, stop=False,
                                             skip_group_check=True,
                                             tile_position=tp)
                r16 = wk2.tile([128, 512], f16, tag="g_r")
                z16 = wk2.tile([128, 512], f16, tag="g_z")
                nc.scalar.activation(out=r16[:, :L], in_=pr[:, :L], func=AF.Sigmoid,
                                     bias=biases[:, 4 * j:4 * j + 1], scale=1.0)
                nc.scalar.activation(out=z16[:, :L], in_=pz[:, :L], func=AF.Sigmoid,
                                     bias=biases[:, 4 * j + 1:4 * j + 2], scale=1.0)
                t16 = wk2.tile([128, 512], f16, tag="g_t")
                nc.vector.scalar_tensor_tensor(
                    out=t16[:, :L], in0=ph[:, :L],
                    scalar=biases[:, 4 * j + 3:4 * j + 4], in1=r16[:, :L],
                    op0=OP.add, op1=OP.mult)
                u16 = wk2.tile([128, 512], f16, tag="g_u")
                nc.vector.tensor_tensor(out=u16[:, :L], in0=t16[:, :L],
                                        in1=pn[:, :L], op=OP.add)
                n16 = wk2.tile([128, 512], f16, tag="g_n")
                nc.scalar.activation(out=n16[:, :L], in_=u16[:, :L], func=AF.Tanh,
                                     bias=biases[:, 4 * j + 2:4 * j + 3], scale=1.0)
                v16 = wk2.tile([128, 512], f16, tag="g_t")
                nc.vector.tensor_tensor(out=v16[:, :L], in0=hT[:, c0:c0 + L],
                                        in1=n16[:, :L], op=OP.subtract)
                w16 = wk2.tile([128, 512], f16, tag="g_u")
                nc.vector.tensor_tensor(out=w16[:, :L], in0=z16[:, :L],
                                        in1=v16[:, :L], op=OP.mult)
                nc.vector.tensor_tensor(out=hT[:, c0:c0 + L], in0=n16[:, :L],
                                        in1=w16[:, :L], op=OP.add)

        # ================= FINAL: y + pooling =================
        for c0, L in PIECES_H:
            py = ps.tile([2, 512], f32, tag="ps")
            nc.tensor.matmul(py[:, :L], wy[:], hT[:, c0:c0 + L],
                             start=True, stop=True)
            ystg = wk2.tile([2, 512], f32, **GGT)
            nc.vector.tensor_copy(out=ystg[:, :L], in_=py[:, :L])
            nc.sync.dma_start(out=y_dram[:, c0:c0 + L], in_=ystg[:, :L])
        ypool = wk.tile([128, SUB], f32, **BUFA)
        for half in range(2):
            src = bass.AP(y_dram.tensor, y_dram[:].offset + half * HALF,
                          [(SUB, 4), (0, 16), (1, SUB)])
            nc.sync.dma_start(out=ypool[:][64 * half:64 * (half + 1)], in_=src)
        ycum = wk.tile([128, SUB], f32, **BUFB)
        nc.vector.tensor_tensor_scan(out=ycum[:], data0=pmask[:],
                                     data1=ypool[:], initial=0.0,
                                     op0=OP.mult, op1=OP.add)
        pooled = wk2.tile([128, GSLOT], f32, tag="g_r")
        nc.gpsimd.ap_gather(out_ap=pooled[:], in_ap=ycum[:], idxs_ap=pidx[:],
                            channels=128, num_elems=SUB, d=1, num_idxs=GSLOT)
        nc.sync.dma_start(out=out_d[:], in_=pooled[:][0::16])

    nc.compile()
    return nc


# ================= driver =================
_CACHE = {}


def kernel(**inputs):
    inputs = {k: np.asarray(v) for k, v in inputs.items()}
    in_maps_data, meta = host_prep(inputs)
    w = fold_weights_host(inputs)
    EP = meta["EP"]
    if EP not in _CACHE:
        _CACHE[EP] = build(EP)
    nc = _CACHE[EP]
    from concourse.bass_utils import run_bass_kernel_spmd
    in_maps = []
    for c in range(NC):
        m = dict(in_maps_data[c])
        m.update(w)
        in_maps.append(m)
    trace = os.environ.get("KERNEL_PROFILE", "0") == "1"
    br = run_bass_kernel_spmd(nc, in_maps, list(range(NC)), trace=trace)
    if trace and br.exec_time_ns is not None:
        print(f"HW exec time: {br.exec_time_ns} ns")
    got = np.zeros(NG, np.float32)
    for c in range(NC):
        pooled = br.results[c]["pooled"]
        for s in range(8):
            for i, (g, endpos) in enumerate(meta["pool_graphs"][c][s]):
                got[g] = pooled[s, i]
    return got



# revision 21
# speedup vs baseline: 1.1711x; 1.1711x over previous
"""Trainium2 Bass kernel for nn_Net_76622216561354 (gnn_message_passing).

Self-contained: host-side sharding/index prep (numpy) + an 8-core SPMD
Bass/Tile kernel run via run_bass_kernel_spmd. Accepts FULL inputs, returns
the FULL pooled output [8192] float32.
"""
import numpy as np
import concourse.bass as bass
import concourse.mybir as mybir
import concourse.tile as tile
from concourse import bacc
from contextlib import ExitStack
import os

import numpy as np

NC = 8
N = 131072; E = 524288; F_IN = 16; DIM = 64; DNN = 16; BK = 4; NG = 8192
NL1 = 4; NL2 = 2
SUB = 2112
NPAD = 8 * SUB          # 16896
HALF = NPAD // 2        # 8448
ECH = 512
GSLOT = 192             # pooled graph slots per sub-chunk (padded)


def host_prep(inputs):
    ei = np.asarray(inputs["edge_index"])
    batch = np.asarray(inputs["batch"]).astype(np.int64)
    src, dst = ei[0].astype(np.int64), ei[1].astype(np.int64)

    # ---- graph spans ----
    # graphs may be empty; gstart[g] = first node of graph g (batch sorted)
    gsizes = np.bincount(batch, minlength=NG)
    gstart = np.concatenate([[0], np.cumsum(gsizes)])

    # ---- core cuts at graph boundaries ----
    cuts = [0]
    for c in range(1, NC):
        t = c * (N // NC)
        while t < N and batch[t] == batch[t - 1]:
            t += 1
        cuts.append(t)
    cuts.append(N)
    cuts = np.array(cuts, np.int64)

    # ---- per-core: pack graphs into 8 graph-aligned sub-chunks ----
    g2l = np.full(N, -1, np.int64)      # global node -> local slot (within its core)
    node_core = np.zeros(N, np.int64)
    l2g = [np.full(NPAD, -1, np.int64) for _ in range(NC)]
    # pooling bookkeeping: per core, per sub-chunk: list of (graph_id, end_pos)
    pool_graphs = [[[] for _ in range(8)] for _ in range(NC)]
    pool_mask = [np.zeros((8, SUB), np.float32) for _ in range(NC)]

    for c in range(NC):
        lo, hi = cuts[c], cuts[c + 1]
        glo, ghi = batch[lo], (batch[hi - 1] + 1 if hi > lo else batch[lo])
        s = 0; pos = 0
        for g in range(glo, ghi):
            sz = int(gsizes[g])
            if sz == 0:
                continue
            if pos + sz > SUB:
                s += 1; pos = 0
                assert s < 8, f"core {c}: sub-chunk overflow"
                assert sz <= SUB
            nodes = np.arange(gstart[g], gstart[g] + sz)
            slots = s * SUB + pos + np.arange(sz)
            g2l[nodes] = slots
            node_core[nodes] = c
            l2g[c][slots] = nodes
            pool_mask[c][s, pos + 1: pos + sz] = 1.0  # same-graph continuation
            # pos of graph end within sub-chunk stream
            pool_graphs[c][s].append((g, pos + sz - 1))
            pos += sz
        assert hi == lo or batch[hi - 1] + 1 == ghi

    # ---- per (core, block) edge streams ----
    # count first to get EP
    counts = np.zeros((NC, NC), np.int64)
    dst_core = node_core[dst]; src_core = node_core[src]
    for c in range(NC):
        for b in range(NC):
            counts[c, b] = np.count_nonzero((dst_core == c) & (src_core == b))
    maxcnt = int(counts.max())
    EP = ((maxcnt + 1 + ECH - 1) // ECH) * ECH
    nchunk = EP // ECH

    dstslot = g2l[dst]; srcslot = g2l[src]

    indeg = np.bincount(dst, minlength=N).astype(np.float64)
    inv = 1.0 / np.maximum(indeg, 1.0)
    ea_all = np.asarray(inputs["edge_attr"]).astype(np.float64)

    per_core = []
    for c in range(NC):
        gidx = np.zeros((8, EP), np.int64)       # src local slot per stream pos
        craw = np.zeros((8, 6, EP), np.float32)  # inv, inv*ea*4, mask (premult)
        ends = np.zeros((8, NPAD), np.int64)
        for b in range(NC):
            m = (dst_core == c) & (src_core == b)
            eids = np.nonzero(m)[0]
            order = np.argsort(dstslot[eids], kind="stable")
            eids = eids[order]
            k = len(eids)
            ps = 1 + np.arange(k)               # positions (0 = dummy)
            gidx[b, ps] = srcslot[eids]
            einv = inv[dst[eids]]
            craw[b, 0, ps] = einv.astype(np.float32)
            for q in range(4):
                craw[b, 1 + q, ps] = (einv * ea_all[eids, q]).astype(np.float32)
            dsl = dstslot[eids]
            same = np.zeros(k, bool)
            if k > 0:
                same[1:] = dsl[1:] == dsl[:-1]
                craw[b, 5, ps] = same.astype(np.float32)
                last = np.zeros(NPAD, np.int64)
                last[dsl] = ps                  # dsl sorted -> last write wins
                ends[b] = last
        per_core.append(dict(gidx=gidx, craw=craw, ends=ends))

    # per-sub-chunk stream-position upper bounds (over all cores/blocks):
    # lets the kernel's ends-gathers read only a prefix of cum
    pend = [0] * 8
    for c in range(NC):
        ends_c = per_core[c]["ends"]
        for ch in range(8):
            m = int(ends_c[:, ch * SUB:(ch + 1) * SUB].max())
            pend[ch] = max(pend[ch], m + 1)
    pend = tuple(min(EP, ((p + 15) // 16) * 16) for p in pend)

    # ---- wrap helper: seq -> [16, L/16] with idx[p, s] = seq[s*16+p] ----
    def wrap16(seq):
        L = len(seq)
        assert L % 16 == 0
        return np.asarray(seq).reshape(L // 16, 16).T.copy()

    ECHUNKS = [4096, 4096, 4096, 4096, 512]
    assert sum(ECHUNKS) == NPAD

    in_maps = []
    for c in range(NC):
        pc = per_core[c]
        # gather idx: [128, EP/16] int16, wrapped per gather call
        # (super-chunks of 4*ECH, remainder as one final call)
        GCH = 6 * ECH
        gidx_t = np.zeros((128, EP // 16), np.int16)
        for b in range(NC):
            off = 0
            while off < EP:
                L = min(GCH, EP - off)
                seq = pc["gidx"][b, off:off + L]
                gidx_t[16 * b:16 * (b + 1), off // 16:(off + L) // 16] = \
                    wrap16(seq).astype(np.int16)
                off += L
        # ends idx: [128, NPAD/16] int16, wrapped per ends-chunk
        eidx_t = np.zeros((128, NPAD // 16), np.int16)
        off = 0
        for L in ECHUNKS:
            for b in range(NC):
                seq = pc["ends"][b, off:off + L]
                eidx_t[16 * b:16 * (b + 1), off // 16:(off + L) // 16] = \
                    wrap16(seq).astype(np.int16)
            off += L
        # c compact (premultiplied, NOT replicated): [8, 6, EP] fp16
        craw8 = pc["craw"].astype(np.float16)
        # pooling mask compact: [8, SUB] fp16
        pmask8 = pool_mask[c].astype(np.float16)
        pidx_t = np.zeros((128, GSLOT // 16), np.int16)
        for s in range(8):
            seq = np.zeros(GSLOT, np.int64)
            gl = pool_graphs[c][s]
            assert len(gl) <= GSLOT, f"GSLOT overflow: {len(gl)}"
            for i, (g, endpos) in enumerate(gl):
                seq[i] = endpos
            pidx_t[16 * s:16 * (s + 1), :] = wrap16(seq).astype(np.int16)
        # x slab transposed [16, NPAD] f16
        xT = np.zeros((16, NPAD), np.float16)
        real = l2g[c] >= 0
        xT[:, real] = np.asarray(inputs["x"])[l2g[c][real]].T.astype(np.float16)
        in_maps.append(dict(xT=xT, gidx=gidx_t, eidx=eidx_t, craw8=craw8,
                            pmask8=pmask8, pidx=pidx_t))

    meta = dict(EP=EP, nchunk=nchunk, ECHUNKS=ECHUNKS, cuts=cuts, pend=pend,
                pool_graphs=pool_graphs, l2g=l2g)
    return in_maps, meta


def fold_weights_host(inputs):
    """float64 weight folds -> shipped stationaries/biases (per-core identical)."""
    dt = np.float64
    lin0_w = np.asarray(inputs["lin0_w"], dt); lin0_b = np.asarray(inputs["lin0_b"], dt)
    lin1_w = np.asarray(inputs["lin1_w"], dt); lin1_b = np.asarray(inputs["lin1_b"], dt)
    lin2_w = np.asarray(inputs["lin2_w"], dt)
    root_w = np.asarray(inputs["root_w"], dt); conv_b = np.asarray(inputs["conv_b"], dt)
    nn1_w = np.asarray(inputs["nn1_w"], dt); nn1_b = np.asarray(inputs["nn1_b"], dt)
    gw_ih = np.asarray(inputs["gru_w_ih"], dt); gw_hh = np.asarray(inputs["gru_w_hh"], dt)
    gb_ih = np.asarray(inputs["gru_b_ih"], dt); gb_hh = np.asarray(inputs["gru_b_hh"], dt)

    Bm = nn1_b.reshape(DNN, DNN)
    Ak = nn1_w.reshape(BK, DNN, DNN)
    M = np.concatenate([Bm[None], Ak], axis=0)            # [5,16,16]

    w = {}
    # compact shipped forms; block-diag / tiled stationaries are expanded
    # on-device with small broadcast DMAs (keeps the relay payload tiny).
    w["lin1w"] = lin1_w.astype(np.float16)                 # [64, 16]
    w["M5"] = M.astype(np.float16)                         # [5, 16, 16]
    whs64 = np.zeros((NL1 * 4, 64, 64), np.float32)
    wfold16 = np.zeros((NL1 * 3, 16, 64), np.float32)
    biases = np.zeros((128, 17), np.float32)
    for j in range(NL1):
        P = lin1_w @ root_w @ gw_ih[j].T                  # [64,192]
        W_rz = P[:, :2 * DIM] + gw_hh[j].T[:, :2 * DIM]
        W_ni = P[:, 2 * DIM:]
        W_nh = gw_hh[j].T[:, 2 * DIM:]
        grp_w = [W_rz[:, :64], W_rz[:, 64:], W_ni, W_nh]
        for g in range(4):
            whs64[4 * j + g] = grp_w[g]
        wihT = gw_ih[j].T                                  # [16,192]
        for g in range(3):
            wfold16[3 * j + g] = wihT[:, 64 * g:64 * (g + 1)]
        b_base = (lin1_b @ root_w + conv_b) @ gw_ih[j].T   # [192]
        b_rz = b_base[:2 * DIM] + gb_ih[j][:2 * DIM] + gb_hh[j][:2 * DIM]
        b_ni = b_base[2 * DIM:] + gb_ih[j][2 * DIM:]
        b_hn = gb_hh[j][2 * DIM:]
        vec = [b_rz[:64], b_rz[64:], b_ni, b_hn]
        for g in range(4):
            biases[0:64, 4 * j + g] = vec[g]
            biases[64:128, 4 * j + g] = vec[g]
    w["whs64"] = whs64.astype(np.float16)
    w["wfold16"] = wfold16.astype(np.float16)
    biases[0:64, 16] = lin0_b
    biases[64:128, 16] = lin0_b
    w["biases"] = biases.astype(np.float32)
    w["lin0c"] = lin0_w.astype(np.float16)                 # [16, 64]
    # y stationary [128, 2] f16
    wy = np.zeros((128, 2), np.float32)
    wy[0:64, 0] = lin2_w[:, 0]
    wy[64:128, 1] = lin2_w[:, 0]
    w["wy"] = wy.astype(np.float16)
    return w


# ================= kernel builder =================

import concourse.bass as bass
import concourse.mybir as mybir
import concourse.tile as tile
from concourse import bacc
from contextlib import ExitStack

NITER = 8

f32 = mybir.dt.float32
f16 = mybir.dt.float16
i16 = mybir.dt.int16
AF = mybir.ActivationFunctionType
OP = mybir.AluOpType


def pieces(total, step):
    out = []
    off = 0
    while off < total:
        out.append((off, min(step, total - off)))
        off += step
    return out


def ends_pieces(c0, L):
    out = []
    while L > 0:
        ch = c0 // SUB
        off = c0 % SUB
        ln = min(L, SUB - off)
        out.append((ch, off, ln))
        c0 += ln
        L -= ln
    return out


def build(EP, fake_collective=False, niter=NITER, pend=None):
    if pend is None:
        pend = [EP] * 8
    nchunk = EP // ECH
    nc = bacc.Bacc("TRN2", target_bir_lowering=False, debug=False, num_devices=NC)

    xT_d = nc.dram_tensor("xT", [16, NPAD], f16, kind="ExternalInput")
    gidx_d = nc.dram_tensor("gidx", [128, EP // 16], i16, kind="ExternalInput")
    eidx_d = nc.dram_tensor("eidx", [128, NPAD // 16], i16, kind="ExternalInput")
    craw8_d = nc.dram_tensor("craw8", [8, 6, EP], f16, kind="ExternalInput")
    pmask8_d = nc.dram_tensor("pmask8", [8, SUB], f16, kind="ExternalInput")
    pidx_d = nc.dram_tensor("pidx", [128, GSLOT // 16], i16, kind="ExternalInput")
    lin1w_d = nc.dram_tensor("lin1w", [64, 16], f16, kind="ExternalInput")
    M5_d = nc.dram_tensor("M5", [5, 16, 16], f16, kind="ExternalInput")
    whs64_d = nc.dram_tensor("whs64", [NL1 * 4, 64, 64], f16, kind="ExternalInput")
    wfold16_d = nc.dram_tensor("wfold16", [NL1 * 3, 16, 64], f16,
                               kind="ExternalInput")
    biases_d = nc.dram_tensor("biases", [128, 17], f32, kind="ExternalInput")
    lin0c_d = nc.dram_tensor("lin0c", [16, 64], f16, kind="ExternalInput")
    wy_d = nc.dram_tensor("wy", [128, 2], f16, kind="ExternalInput")
    out_d = nc.dram_tensor("pooled", [8, GSLOT], f32, kind="ExternalOutput")

    PIECES_H = pieces(HALF, 512)
    PIECES_S = pieces(SUB, 512)

    with tile.TileContext(nc) as tc, ExitStack() as ex:
        pp = ex.enter_context(tc.tile_pool(name="persist", bufs=1))
        wk = ex.enter_context(tc.tile_pool(name="work", bufs=2))
        wk2 = ex.enter_context(tc.tile_pool(name="work2", bufs=2))
        eb1 = ex.enter_context(tc.tile_pool(name="ebc", bufs=1))
        ps = ex.enter_context(tc.tile_pool(name="psum", bufs=8, space="PSUM"))
        dr = ex.enter_context(tc.tile_pool(name="dram", bufs=1, space="DRAM"))

        BUFA = dict(tag="bufA")   # >= 8.25KB slots
        BUFB = dict(tag="bufB")
        GGT = dict(tag="gg")      # 2KB slots

        hT = pp.tile([128, HALF], f16, tag="hT")
        table = pp.tile([128, NPAD], f32, tag="table")
        cum = pp.tile([128, EP], f32, tag="cum")
        nc.vector.memset(cum[:], 0)
        gidx = pp.tile([128, EP // 16], i16, tag="gidx")
        eidx = pp.tile([128, NPAD // 16], i16, tag="eidx")
        pmask = pp.tile([128, SUB], f16, tag="pmask")
        pidx = pp.tile([128, GSLOT // 16], i16, tag="pidx")
        biases = pp.tile([128, 17], f32, tag="biases")
        wy = pp.tile([128, 2], f16, tag="wy")

        nc.sync.dma_start(out=gidx[:], in_=gidx_d[:])
        nc.sync.dma_start(out=eidx[:], in_=eidx_d[:])
        # pmask broadcast-expand [8,SUB] -> [128,SUB] (each row 16x)
        nc.sync.dma_start(
            out=pmask[:], in_=bass.AP(pmask8_d, 0, [(SUB, 8), (0, 16), (1, SUB)]))
        nc.sync.dma_start(out=pidx[:], in_=pidx_d[:])
        nc.sync.dma_start(out=biases[:], in_=biases_d[:])
        nc.sync.dma_start(out=wy[:], in_=wy_d[:])

        # ---- stationaries expanded on-device from compact shipped forms ----
        # wslab: block b holds lin1_w at partitions 64*(b//4).., cols 16b..
        wslab_s = pp.tile([128, 8, 128], f16, tag="wslab_s")
        nc.vector.memset(wslab_s[:], 0)
        PS = wslab_s[:].ap[0][0]
        for grp in range(2):
            dst = bass.AP(wslab_s.tensor,
                          wslab_s[:].offset + grp * (64 * PS + 4 * 128 + 64),
                          [(PS, 64), (128 + 16, 4), (1, 16)])
            nc.sync.dma_start(
                out=dst, in_=bass.AP(lin1w_d, 0, [(16, 64), (0, 4), (1, 16)]))
        # wM: block-diag-8 of M[p] per stream p
        wM_s = pp.tile([128, 5, 128], f16, tag="wM_s")
        nc.vector.memset(wM_s[:], 0)
        PSM = wM_s[:].ap[0][0]
        for b in range(8):
            dst = bass.AP(wM_s.tensor, wM_s[:].offset + 16 * b * PSM + 16 * b,
                          [(PSM, 16), (128, 5), (1, 16)])
            nc.sync.dma_start(
                out=dst, in_=bass.AP(M5_d, 0, [(16, 16), (256, 5), (1, 16)]))
        # whs: two diagonal 64x64 copies per (j,g)
        whs_s = pp.tile([128, NL1 * 4, 128], f16, tag="whs_s")
        nc.vector.memset(whs_s[:], 0)
        PSH = whs_s[:].ap[0][0]
        for h in range(2):
            dst = bass.AP(whs_s.tensor, whs_s[:].offset + h * (64 * PSH + 64),
                          [(PSH, 64), (128, 16), (1, 64)])
            nc.sync.dma_start(
                out=dst, in_=bass.AP(whs64_d, 0, [(64, 64), (4096, 16), (1, 64)]))
        # wfold: [16,64] block tiled 8x down the partitions
        wfold_s = pp.tile([128, NL1 * 3, 64], f16, tag="wfold_s")
        PSF = wfold_s[:].ap[0][0]
        for b in range(8):
            dst = bass.AP(wfold_s.tensor, wfold_s[:].offset + 16 * b * PSF,
                          [(PSF, 16), (64, 12), (1, 64)])
            nc.sync.dma_start(
                out=dst, in_=bass.AP(wfold16_d, 0, [(64, 16), (1024, 12), (1, 64)]))

        slab_dram = dr.tile([128, SUB], f16)
        ag_dram = dr.tile([NC, 128, SUB], f16)
        cexp_dram = dr.tile([nchunk, 128, 6 * ECH], f16)
        y_dram = dr.tile([2, HALF], f16)

        # ================= INIT =================
        # expand compact craw8 [8,6,EP] -> chunked cexp_dram [nchunk,128,6*ECH]
        # (DRAM->DRAM, each block row duplicated over its 16 partitions)
        for k in range(nchunk):
            for s in range(6):
                dst = bass.AP(cexp_dram.tensor,
                              cexp_dram[:].offset + k * (128 * 6 * ECH) + s * ECH,
                              [(6 * ECH, 128), (1, ECH)])
                src = bass.AP(craw8_d, s * EP + k * ECH,
                              [(6 * EP, 8), (0, 16), (1, ECH)])
                nc.sync.dma_start(out=dst, in_=src)

        # lin0 -> hT
        wlin0 = pp.tile([16, 2, 128], f16, tag="wlin0")
        nc.vector.memset(wlin0[:], 0)
        PSL = wlin0[:].ap[0][0]
        for h in range(2):
            dst = bass.AP(wlin0.tensor, wlin0[:].offset + h * (128 + 64),
                          [(PSL, 16), (1, 64)])
            nc.sync.dma_start(
                out=dst, in_=bass.AP(lin0c_d, 0, [(64, 16), (1, 64)]))
        for c0, L in PIECES_H:
            xa = wk2.tile([16, 512], f16, **GGT)
            nc.sync.dma_start(out=xa[:, :L], in_=xT_d[:, c0:c0 + L])
            xb = wk2.tile([16, 512], f16, **GGT)
            nc.sync.dma_start(out=xb[:, :L], in_=xT_d[:, HALF + c0:HALF + c0 + L])
            p0 = ps.tile([128, 512], f32, tag="ps")
            nc.tensor.matmul(p0[:, :L], wlin0[:, 0, :], xa[:, :L],
                             start=True, stop=False)
            nc.tensor.matmul(p0[:, :L], wlin0[:, 1, :], xb[:, :L],
                             start=False, stop=True)
            nc.scalar.activation(out=hT[:, c0:c0 + L], in_=p0[:, :L],
                                 func=AF.Relu, bias=biases[:, 16:17], scale=1.0)

        # ================= ITERATIONS =================
        for it in range(niter):
            j = (it // 2) % NL1
            # ---- A: slab + exchange ----
            for c0, L in PIECES_S:
                p0 = ps.tile([128, 512], f32, tag="ps")
                for b in range(8):
                    rc0 = (b % 4) * SUB + c0
                    nc.tensor.matmul(p0[:, :L], wslab_s[:, b, :],
                                     hT[:, rc0:rc0 + L],
                                     start=(b == 0), stop=(b == 7))
                stg = wk2.tile([128, 512], f16, tag="slabstg")
                nc.vector.tensor_copy(out=stg[:, :L], in_=p0[:, :L])
                nc.sync.dma_start(out=slab_dram[:, c0:c0 + L],
                                  in_=stg[:, :L])
            if fake_collective:
                for cc_ in range(NC):
                    nc.sync.dma_start(out=ag_dram[cc_], in_=slab_dram[:])
            else:
                nc.gpsimd.collective_compute(
                    "AllGather", OP.bypass,
                    replica_groups=[list(range(NC))],
                    ins=[slab_dram[:].opt()], outs=[ag_dram[:].opt()])
            HSUB = SUB // 4
            for s in range(8):
                for h2 in range(4):
                    tstg = wk2.tile([128, HSUB], f16, tag="tstg")
                    src = bass.AP(ag_dram.tensor,
                                  ag_dram[:].offset + (16 * s) * SUB + h2 * HSUB,
                                  [(128 * SUB, 8), (SUB, 16), (1, HSUB)])
                    nc.sync.dma_start(out=tstg[:], in_=src)
                    dstt = bass.AP(table.tensor,
                                   table[:].offset + s * SUB + h2 * HSUB,
                                   [table[:].ap[0], (1, HSUB)])
                    nc.scalar.activation(out=dstt, in_=tstg[:], func=AF.Copy,
                                         scale=1.0)

            # ---- C: edge chunks (gathers batched 6x to amortize ap_gather;
            # chunks processed in pairs: mults first, then matmuls, then
            # scans, so the DVE does not head-of-line block on PE) ----
            GCH = 6 * ECH
            gbuf = None
            for k0 in range(0, nchunk, 2):
                kpair = [k for k in (k0, k0 + 1) if k < nchunk]
                ccs, scs, msgs = {}, {}, {}
                for k in kpair:
                    cc = wk.tile([128, 6, ECH], f16, **BUFA)
                    nc.sync.dma_start(out=cc[:], in_=cexp_dram[k])
                    ccs[k] = cc
                    if k % 6 == 0:
                        G0 = k * ECH
                        GL = min(GCH, EP - G0)
                        gbuf = pp.tile([128, GCH], f32, tag="gbuf")
                        nc.gpsimd.ap_gather(
                            out_ap=gbuf[:, :GL], in_ap=table[:],
                            idxs_ap=gidx[:, G0 // 16:(G0 + GL) // 16],
                            channels=128, num_elems=NPAD, d=1, num_idxs=GL)
                    sc = wk.tile([128, 5, ECH], f16, **BUFB)
                    g_in0 = bass.AP(gbuf.tensor,
                                    gbuf[:].offset + (k % 6) * ECH,
                                    [gbuf[:].ap[0], (0, 5), (1, ECH)])
                    nc.vector.tensor_tensor(out=sc[:], in0=g_in0,
                                            in1=cc[:, 0:5, :], op=OP.mult)
                    scs[k] = sc
                for k in kpair:
                    msg = ps.tile([128, 512], f32, tag="ps")
                    for p in range(5):
                        nc.tensor.matmul(msg[:, :ECH], wM_s[:, p, :],
                                         scs[k][:, p, :],
                                         start=(p == 0), stop=(p == 4))
                    msgs[k] = msg
                for k in kpair:
                    cum_out = bass.AP(cum.tensor, cum[:].offset + k * ECH,
                                      [cum[:].ap[0], (1, ECH)])
                    if k == 0:
                        init = 0.0
                    else:
                        init = bass.AP(cum.tensor,
                                       cum[:].offset + (k * ECH - 1),
                                       [cum[:].ap[0], (1, 1)])
                    nc.vector.tensor_tensor_scan(out=cum_out,
                                                 data0=ccs[k][:, 5, :],
                                                 data1=msgs[k][:, :ECH],
                                                 initial=init,
                                                 op0=OP.mult, op1=OP.add)

            # ---- D+E: ends + gates ----
            ends_tiles = {}

            def get_ends(ch, _et=ends_tiles):
                if ch in _et:
                    return _et[ch]
                ebf = wk.tile([128, SUB], f32, **(BUFA if ch < 4 else BUFB))
                P = pend[ch]
                nc.gpsimd.ap_gather(
                    out_ap=ebf[:], in_ap=cum[:, :P],
                    idxs_ap=eidx[:, ch * (SUB // 16):(ch + 1) * (SUB // 16)],
                    channels=128, num_elems=P, d=1, num_idxs=SUB)
                eb = eb1.tile([128, SUB], f16, tag=("ebA" if ch < 4 else "ebB"))
                nc.scalar.activation(out=eb[:], in_=ebf[:], func=AF.Copy,
                                     scale=1.0)
                for o in list(_et):
                    if o != ch and (o < 4) == (ch < 4):
                        del _et[o]
                _et[ch] = eb
                return eb

            for c0, L in PIECES_H:
                pr = ps.tile([128, 512], f32, tag="ps")
                pz = ps.tile([128, 512], f32, tag="ps")
                pn = ps.tile([128, 512], f32, tag="ps")
                ph = ps.tile([128, 512], f32, tag="ps")
                for g, pst in enumerate([pr, pz, pn, ph]):
                    nc.tensor.matmul(pst[:, :L], whs_s[:, 4 * j + g, :],
                                     hT[:, c0:c0 + L],
                                     start=True, stop=True)
                # chunk-major over ends pieces, gates inner: keeps each ends
                # chunk gathered exactly once per iteration (no cache thrash
                # at sub-chunk-crossing pieces)
                for half in range(2):
                    pcs = ends_pieces(half * HALF + c0, L)
                    for i, (ch, eoff, eln) in enumerate(pcs):
                        eb = get_ends(ch)
                        rhs = bass.AP(eb.tensor, eb[:].offset + eoff,
                                      [eb[:].ap[0], (1, eln)])
                        oo = (eoff + ch * SUB) - (half * HALF + c0)
                        tp = (0, 64) if half == 1 else None
                        for g, pst in enumerate([pr, pz, pn]):
                            out = bass.AP(
                                pst.tensor,
                                pst[:].offset + 64 * half * pst[:].ap[0][0] + oo,
                                [(pst[:].ap[0][0], 64), (1, eln)])
                            nc.tensor.matmul(out, wfold_s[:, 3 * j + g, :], rhs,
                                             start=False, stop=False,
                                             skip_group_check=True,
                                             tile_position=tp)
                r16 = wk2.tile([128, 512], f16, tag="g_r")
                z16 = wk2.tile([128, 512], f16, tag="g_z")
                nc.scalar.activation(out=r16[:, :L], in_=pr[:, :L], func=AF.Sigmoid,
                                     bias=biases[:, 4 * j:4 * j + 1], scale=1.0)
                nc.scalar.activation(out=z16[:, :L], in_=pz[:, :L], func=AF.Sigmoid,
                                     bias=biases[:, 4 * j + 1:4 * j + 2], scale=1.0)
                t16 = wk2.tile([128, 512], f16, tag="g_t")
                nc.vector.scalar_tensor_tensor(
                    out=t16[:, :L], in0=ph[:, :L],
                    scalar=biases[:, 4 * j + 3:4 * j + 4], in1=r16[:, :L],
                    op0=OP.add, op1=OP.mult)
                u16 = wk2.tile([128, 512], f16, tag="g_u")
                nc.vector.tensor_tensor(out=u16[:, :L], in0=t16[:, :L],
                                        in1=pn[:, :L], op=OP.add)
                n16 = wk2.tile([128, 512], f16, tag="g_n")
                nc.scalar.activation(out=n16[:, :L], in_=u16[:, :L], func=AF.Tanh,
                                     bias=biases[:, 4 * j + 2:4 * j + 3], scale=1.0)
                v16 = wk2.tile([128, 512], f16, tag="g_t")
                nc.vector.tensor_tensor(out=v16[:, :L], in0=hT[:, c0:c0 + L],
                                        in1=n16[:, :L], op=OP.subtract)
                w16 = wk2.tile([128, 512], f16, tag="g_u")
                nc.vector.tensor_tensor(out=w16[:, :L], in0=z16[:, :L],
                                        in1=v16[:, :L], op=OP.mult)
                nc.vector.tensor_tensor(out=hT[:, c0:c0 + L], in0=n16[:, :L],
                                        in1=w16[:, :L], op=OP.add)

        # ================= FINAL: y + pooling =================
        for c0, L in PIECES_H:
            py = ps.tile([2, 512], f32, tag="ps")
            nc.tensor.matmul(py[:, :L], wy[:], hT[:, c0:c0 + L],
                             start=True, stop=True)
            ystg = wk2.tile([2, 512], f16, **GGT)
            nc.vector.tensor_copy(out=ystg[:, :L], in_=py[:, :L])
            nc.sync.dma_start(out=y_dram[:, c0:c0 + L], in_=ystg[:, :L])
        ypool = wk.tile([128, SUB], f16, **BUFA)
        for half in range(2):
            src = bass.AP(y_dram.tensor, y_dram[:].offset + half * HALF,
                          [(SUB, 4), (0, 16), (1, SUB)])
            nc.sync.dma_start(out=ypool[:][64 * half:64 * (half + 1)], in_=src)
        ycum = wk.tile([128, SUB], f32, **BUFB)
        nc.vector.tensor_tensor_scan(out=ycum[:], data0=pmask[:],
                                     data1=ypool[:], initial=0.0,
                                     op0=OP.mult, op1=OP.add)
        pooled = wk2.tile([128, GSLOT], f32, tag="g_r")
        nc.gpsimd.ap_gather(out_ap=pooled[:], in_ap=ycum[:], idxs_ap=pidx[:],
                            channels=128, num_elems=SUB, d=1, num_idxs=GSLOT)
        nc.sync.dma_start(out=out_d[:], in_=pooled[:][0::16])

    nc.compile()
    return nc


# ================= driver =================
_CACHE = {}


def kernel(**inputs):
    inputs = {k: np.asarray(v) for k, v in inputs.items()}
    in_maps_data, meta = host_prep(inputs)
    w = fold_weights_host(inputs)
    EP = meta["EP"]
    key = (EP, meta["pend"])
    if key not in _CACHE:
        _CACHE[key] = build(EP, pend=meta["pend"])
    nc = _CACHE[key]
    from concourse.bass_utils import run_bass_kernel_spmd
    in_maps = []
    for c in range(NC):
        m = dict(in_maps_data[c])
        m.update(w)
        in_maps.append(m)
    trace = os.environ.get("KERNEL_PROFILE", "0") == "1"
    br = run_bass_kernel_spmd(nc, in_maps, list(range(NC)), trace=trace)
    if trace and br.exec_time_ns is not None:
        print(f"HW exec time: {br.exec_time_ns} ns")
    got = np.zeros(NG, np.float32)
    for c in range(NC):
        pooled = br.results[c]["pooled"]
        for s in range(8):
            for i, (g, endpos) in enumerate(meta["pool_graphs"][c][s]):
                got[g] = pooled[s, i]
    return got

